# revision 1
# baseline (speedup 1.0000x reference)
"""Causal Gaussian-kernel self-attention on 8 TRN2 NeuronCores.

Reference computation (per batch b):
    qkv = x @ W_attn + b_attn ; q,k,v heads of 64 dims
    scores = exp(-(|q|^2 + |k|^2 - 2 q.k) / (2*sqrt(64))), causal-masked, NO softmax
    y = scores @ v ; out = y @ W_proj + b_proj

Sharding: core c -> batch b = c//2, head-group g = c%2 (8 heads each).
Per core the score factors:  exp(q.k/8) * exp(-|q|^2/16) * exp(-|k|^2/16)
  - exp(-|k|^2/16) is folded into v (per-key scale, from a k-natural GEMM)
  - exp(-|q|^2/16) is folded into the y^T PSUM->SBUF copy (per-query scale)
so the TxT map is a single exp() of one matmul result (ACT engine), which is
the ~150us/core floor; everything else is scheduled around it.

Layouts (per core):
  xT      (1024,2048) x[b]^T (host-transposed), fp32r, resident
  q_pack  4x(128,2048) head-pair q^T rows, values -2*(x@Wq+bq), fp32r
  k_pack  4x(128,2048) head-pair k^T rows, fp32r
  v~      16x(128,512) v natural * exp(-|k|^2/16), bf16
  s^T     per k-tile (128 k-rows, q-extent) exact-causal, exp'd to bf16
  y^T     4x(128,2048) head-pair packed, bf16
Row/col tile_position packs both heads of a pair into the PE array
concurrently (K=64 scores at row 0/64; M=64 AV at col 0/64).

Emission order: k-natural GEMM, pair-0 q/k GEMM, v GEMM, qk GEMMs for pairs
1-3, then attention at elevated priority (half-major over pairs, c_proj for
each T-half right after its folds). The scheduler overlaps the remaining
GEMMs and proj under the ACT-bound exp stream. One unified PSUM pool of
4x(128,1024) slots serves every phase so no pool-zone barriers serialize it.

Host side: the two head-group cores of one batch are summed (the c_proj
row-parallel all-reduce) + b_proj.
"""

import math
import os
from contextlib import ExitStack

import numpy as np
import ml_dtypes

import concourse.bass as bass
import concourse.mybir as mybir
import concourse.tile as tile
from concourse.vector_clock import ScopedClock, VectorClock
from concourse.bass_utils import run_bass_kernel_spmd

F32 = mybir.dt.float32
F32R = mybir.dt.float32r
BF16 = mybir.dt.bfloat16
AF = mybir.ActivationFunctionType
ALU = mybir.AluOpType

B, T, C, H = 4, 2048, 1024, 16
HD = C // H          # 64
HG = H // 2          # 8 heads per core
GC = HG * HD         # 512
NT = T // 128        # 16
NKC = C // 128       # 8
SCALE = -1.0 / (2.0 * math.sqrt(HD))   # -1/16

LAST_RESULTS = None
_last_in_maps = None


class _TC(tile.TileContext):
    """Tail barrier emitting one NOP per proc tick; this walrus build
    accepts only a single sync wait per instruction."""

    def _drain_and_barrier(self, tick_clock, wait_clock):
        gc = tick_clock.global_clock
        for proc in range(len(gc)):
            if gc[proc] <= 0:
                continue
            vc = VectorClock()
            vc.require_at_least(proc, gc[proc])
            nop_inst = self.nc.sync.nop(nofuse=True)
            wait_clock.add_sem_waits(nop_inst.ins, ScopedClock({None: vc}))
        self.nc.sync.drain()
        self.nc.all_engine_barrier()
        assert self.sems is not None
        popped = self.nc._tile_sem_poison_stack.pop()
        assert popped is self._sem_poison
        self.nc.clear_and_free_semaphores(list(self.sems.allocated().values()))
        self.nc.all_engine_barrier()


def _split_sync_waits(nc, keep=1):
    """Move excess per-instruction sem waits onto NOPs inserted just before,
    same engine stream (walrus here rejects >1 sync wait per instruction)."""
    for f in nc.m.functions:
        for bb in f.blocks:
            out = []
            changed = False
            for inst in bb.instructions:
                si = inst.sync_info
                waits = list(si.on_wait) if (si is not None and si.on_wait) else []
                if len(waits) > keep:
                    changed = True
                    for w in waits[:-keep]:
                        nop = mybir.InstNoOp(
                            name=f"I-wsplit-{nc.next_id()}", ins=[], outs=[]
                        )
                        nop.engine = inst.engine
                        nop.sync_info = mybir.SyncInfo(on_wait=[w], on_update=[])
                        out.append(nop)
                    ups = list(si.on_update) if si.on_update else []
                    inst.sync_info = mybir.SyncInfo(
                        on_wait=waits[-keep:], on_update=ups
                    )
                out.append(inst)
            if changed:
                bb.instructions = out


def _build_program():
    nc = bass.Bass(target_bir_lowering=False, trn_type="TRN2", debug=False)

    xT_d = nc.dram_tensor("xT", [C, T], F32R, kind="ExternalInput").ap()
    Wq_d = nc.dram_tensor("Wq", [C, GC], F32R, kind="ExternalInput").ap()
    Wk_d = nc.dram_tensor("Wk", [C, GC], F32R, kind="ExternalInput").ap()
    Wv_d = nc.dram_tensor("Wv", [C, GC], F32R, kind="ExternalInput").ap()
    bqc_d = nc.dram_tensor("bq_col", [128, 4], F32, kind="ExternalInput").ap()
    bkc_d = nc.dram_tensor("bk_col", [128, 4], F32, kind="ExternalInput").ap()
    bk_d = nc.dram_tensor("bk", [1, GC], F32R, kind="ExternalInput").ap()
    bv_d = nc.dram_tensor("bv", [1, GC], F32R, kind="ExternalInput").ap()
    Wp_d = nc.dram_tensor("Wp", [GC, C], BF16, kind="ExternalInput").ap()
    mask_d = nc.dram_tensor("trimask", [128, 128], BF16, kind="ExternalInput").ap()
    out_d = nc.dram_tensor("out", [T, C], F32, kind="ExternalOutput").ap()

    with _TC(nc) as tc, ExitStack() as ctx:
        res = ctx.enter_context(tc.tile_pool(name="res", bufs=1))
        ps = ctx.enter_context(tc.tile_pool(name="ps", bufs=4, space="PSUM"))

        def big(nm):
            return ps.tile([128, 1024], F32, tag="big", name=nm)

        # ---- resident small tensors ----
        mask = res.tile([128, 128], BF16, tag="mask")
        nc.sync.dma_start(mask[:], mask_d[:])
        bqc = res.tile([128, 4], F32, tag="bqc")
        nc.sync.dma_start(bqc[:], bqc_d[:])
        bkc = res.tile([128, 4], F32, tag="bkc")
        nc.sync.dma_start(bkc[:], bkc_d[:])
        bk = res.tile([1, GC], F32R, tag="bk")
        nc.sync.dma_start(bk[:], bk_d[:])
        bv = res.tile([1, GC], F32R, tag="bv")
        nc.sync.dma_start(bv[:], bv_d[:])
        ones = res.tile([1, 512], F32R, tag="ones")
        nc.vector.memset(ones[:].bitcast(F32), 1.0)
        ones_q = res.tile([128, 128], F32R, tag="ones_q")
        nc.vector.memset(ones_q[:].bitcast(F32), 0.25)

        # ---- resident big tensors ----
        xt_pool = ctx.enter_context(tc.tile_pool(name="xt", bufs=1))
        xT = []
        for kc in range(NKC):
            xt_t = xt_pool.tile([128, T], F32R, tag=f"x{kc}", name=f"xT{kc}")
            nc.sync.dma_start(xt_t[:], xT_d[kc * 128:(kc + 1) * 128, :])
            xT.append(xt_t)
        qk = ctx.enter_context(tc.tile_pool(name="qk", bufs=1))
        q_pack = [qk.tile([128, T], F32R, tag=f"q{p}", name=f"q_pack{p}")
                  for p in range(4)]
        k_pack = [qk.tile([128, T], F32R, tag=f"k{p}", name=f"k_pack{p}")
                  for p in range(4)]
        vt = ctx.enter_context(tc.tile_pool(name="vt", bufs=1))
        v_sb = [vt.tile([128, GC], BF16, tag=f"v{t}", name=f"v_sb{t}")
                for t in range(NT)]
        yp = ctx.enter_context(tc.tile_pool(name="yp", bufs=1))
        y_sb = [yp.tile([128, T], BF16, tag=f"y{p}", name=f"y_sb{p}")
                for p in range(4)]
        wpp = ctx.enter_context(tc.tile_pool(name="wpp", bufs=1))
        wp = [wpp.tile([128, C], BF16, tag=f"wp{p}", name=f"wp{p}")
              for p in range(4)]
        for p in range(4):
            nc.sync.dma_start(wp[p][:], Wp_d[p * 128:(p + 1) * 128, :])

        # ======== k-natural GEMM -> f_col = exp(scale*|k|^2) ========
        with tc.tile_pool(name="wr", bufs=1) as wr, \
             tc.tile_pool(name="sq2", bufs=2) as sq2, \
             tc.tile_pool(name="fcolp", bufs=1) as fcol_p, \
             tc.tile_pool(name="wst", bufs=4) as wst:
            f_col = [fcol_p.tile([128, HG], F32, tag=f"f{t}", name=f"f_col{t}")
                     for t in range(NT)]
            wk_r = [wr.tile([128, GC], F32R, tag=f"wr{kc}", name=f"wkr{kc}")
                    for kc in range(NKC)]
            for kc in range(NKC):
                nc.sync.dma_start(wk_r[kc][:], Wk_d[kc * 128:(kc + 1) * 128, :])

            for tt in range(NT):
                pkn = big(f"pkn{tt}")
                nc.tensor.matmul(pkn[:, 0:GC], ones[0:1, 0:128], bk[0:1, :],
                                 start=True, stop=False)
                for kc in range(NKC):
                    nc.tensor.matmul(pkn[:, 0:GC],
                                     xT[kc][:, tt * 128:tt * 128 + 128],
                                     wk_r[kc][:], start=False,
                                     stop=(kc == NKC - 1))
                sq_t = sq2.tile([128, GC], F32, tag="sqk", name=f"sqk{tt}")
                nc.scalar.square(sq_t[:], pkn[:, 0:GC])
                k2_t = sq2.tile([128, HG], F32, tag="k2", name=f"k2{tt}")
                nc.vector.tensor_reduce(
                    k2_t[:], sq_t.rearrange("p (s d) -> p s d", s=HG),
                    axis=mybir.AxisListType.X, op=ALU.add)
                nc.scalar.activation(f_col[tt][:], k2_t[:], AF.Exp, scale=SCALE)

            # ======== q^T/k^T GEMMs (emit pair 0 first, rest later) ========
            def emit_qk(p8):
                w_d = Wq_d if p8 < 4 else Wk_d
                bcol = bqc if p8 < 4 else bkc
                m0 = 128 * (p8 % 4)
                dst = q_pack[p8 % 4] if p8 < 4 else k_pack[p8 % 4]
                bigs = [big(f"psqk{p8}_{n2}") for n2 in range(2)]
                wts = []
                for kc in range(NKC):
                    wti = wst.tile([128, 128], F32R, tag="w", name=f"w{p8}_{kc}")
                    nc.sync.dma_start(
                        wti[:], w_d[kc * 128:(kc + 1) * 128, m0:m0 + 128])
                    for n in range(4):
                        nc.tensor.matmul(
                            bigs[n // 2][:, (n % 2) * 512:(n % 2) * 512 + 512],
                            wti[:], xT[kc][:, n * 512:(n + 1) * 512],
                            start=(kc == 0), stop=(kc == NKC - 1))
                for n2 in range(2):
                    nc.vector.tensor_scalar_add(
                        dst[:, n2 * 1024:(n2 + 1) * 1024], bigs[n2][:],
                        bcol[:, (p8 % 4):(p8 % 4) + 1])

            emit_qk(0)
            emit_qk(4)
            prio_mark = tc.cur_priority  # attention slots in here, before v

            # ======== v-natural GEMM, fold f_col, cast bf16 ========
            wv_r = [wr.tile([128, GC], F32R, tag=f"wr{kc}", name=f"wvr{kc}")
                    for kc in range(NKC)]
            for kc in range(NKC):
                nc.sync.dma_start(wv_r[kc][:], Wv_d[kc * 128:(kc + 1) * 128, :])
            for tt in range(NT):
                pv = big(f"pv{tt}")
                nc.tensor.matmul(pv[:, 0:GC], ones[0:1, 0:128], bv[0:1, :],
                                 start=True, stop=False)
                for kc in range(NKC):
                    nc.tensor.matmul(pv[:, 0:GC],
                                     xT[kc][:, tt * 128:tt * 128 + 128],
                                     wv_r[kc][:], start=False,
                                     stop=(kc == NKC - 1))
                fb = f_col[tt]
                fb_b = bass.AP(fb.tensor, fb.offset,
                               [list(fb.ap[0]), [fb.ap[1][0], HG], [0, HD]])
                nc.vector.tensor_tensor(
                    v_sb[tt].rearrange("p (s d) -> p s d", s=HG),
                    pv[:, 0:GC].rearrange("p (s d) -> p s d", s=HG),
                    fb_b, op=ALU.mult)

            for p8 in (1, 5, 2, 6, 3, 7):
                emit_qk(p8)

        # ================= attention (elevated priority) =================
        with tc.tile_pool(name="q2ep", bufs=1) as q2e_p, \
             tc.tile_pool(name="sqq", bufs=2) as sqq, \
             tc.tile_pool(name="ssb", bufs=2) as ssb, \
             tc.tile_pool(name="osb", bufs=2) as osb:
            q2eh = [q2e_p.tile([128, 1024], BF16, tag=f"e{p}", name=f"q2e{p}")
                    for p in range(4)]

            with tc.high_priority(offset=max(0, tc.cur_priority - prio_mark)):
                for half in range(2):
                    q_lo, q_hi = 1024 * half, 1024 * (half + 1)
                    for p in range(4):
                        # q2e for this (pair, half)
                        sq_q = sqq.tile([128, 1024], F32R, tag="sqq",
                                        name=f"sqq{p}_{half}")
                        nc.vector.tensor_mul(sq_q[:], q_pack[p][:, q_lo:q_hi],
                                             q_pack[p][:, q_lo:q_hi])
                        for hh in range(2):
                            pq2 = big(f"pq2_{p}{half}{hh}")
                            for j in range(2):
                                nc.tensor.matmul(
                                    pq2[:, j * 512:(j + 1) * 512],
                                    ones_q[hh * 64:hh * 64 + 64, :],
                                    sq_q[hh * 64:hh * 64 + 64,
                                         j * 512:(j + 1) * 512],
                                    start=True, stop=True,
                                    tile_position=(hh * 64, 0))
                            nc.scalar.activation(
                                q2eh[p][hh * 64:hh * 64 + 64, :],
                                pq2[hh * 64:hh * 64 + 64, :],
                                AF.Exp, scale=SCALE)

                        y_ps = big(f"yps{p}_{half}")
                        kt_last = 8 * half + 7
                        for kt in range(kt_last + 1):
                            q0 = max(128 * kt, q_lo)
                            ext = q_hi - q0
                            for hh in range(2):
                                h = 2 * p + hh
                                s_ps = big(f"sps{p}_{half}_{kt}_{hh}")
                                n0 = q0
                                while n0 < q_hi:
                                    nn = min(512, q_hi - n0)
                                    nc.tensor.matmul(
                                        s_ps[:, n0 - q0:n0 - q0 + nn],
                                        k_pack[p][hh * 64:hh * 64 + 64,
                                                  kt * 128:kt * 128 + 128],
                                        q_pack[p][hh * 64:hh * 64 + 64,
                                                  n0:n0 + nn],
                                        start=True, stop=True,
                                        tile_position=(hh * 64, 0))
                                    n0 += nn
                                s_sb = ssb.tile([128, 1024], BF16, tag=f"s{hh}",
                                                name=f"ssb{p}_{half}_{kt}_{hh}")
                                nc.scalar.activation(s_sb[:, 0:ext],
                                                     s_ps[:, 0:ext],
                                                     AF.Exp, scale=SCALE)
                                if 128 * kt >= q_lo:
                                    nc.vector.tensor_mul(s_sb[:, 0:128],
                                                         s_sb[:, 0:128],
                                                         mask[:])
                                a0 = q0
                                while a0 < q_hi:
                                    a1 = min((a0 // 512 + 1) * 512, q_hi)
                                    nc.tensor.matmul(
                                        y_ps[hh * 64:hh * 64 + 64,
                                             a0 - q_lo:a1 - q_lo],
                                        v_sb[kt][:, h * HD:h * HD + HD],
                                        s_sb[:, a0 - q0:a1 - q0],
                                        start=(kt == 0), stop=(kt == kt_last),
                                        tile_position=(0, hh * 64))
                                    a0 = a1
                        nc.vector.tensor_tensor(
                            y_sb[p][:, q_lo:q_hi], y_ps[:],
                            q2eh[p][:], op=ALU.mult)

                    # ---- c_proj for this T-half ----
                    for tt in range(8 * half, 8 * half + 8):
                        po = big(f"po{tt}")
                        for n2 in range(2):
                            for p4 in range(4):
                                nc.tensor.matmul(
                                    po[:, n2 * 512:(n2 + 1) * 512],
                                    y_sb[p4][:, tt * 128:tt * 128 + 128],
                                    wp[p4][:, n2 * 512:(n2 + 1) * 512],
                                    start=(p4 == 0), stop=(p4 == 3))
                        o_sb = osb.tile([128, C], F32, tag="o", name=f"osb{tt}")
                        nc.vector.tensor_copy(o_sb[:], po[:])
                        nc.sync.dma_start(out_d[tt * 128:(tt + 1) * 128, :],
                                          o_sb[:])

    _split_sync_waits(nc)
    return nc


_NC_CACHE = None


def _get_program():
    global _NC_CACHE
    if _NC_CACHE is None:
        _NC_CACHE = _build_program()
    return _NC_CACHE


def kernel(x, W_attn, b_attn, W_proj, b_proj, n_head):
    global LAST_RESULTS, _last_in_maps
    assert int(n_head) == H
    x = np.asarray(x, dtype=np.float32)
    W_attn = np.asarray(W_attn, dtype=np.float32)
    b_attn = np.asarray(b_attn, dtype=np.float32)
    W_proj = np.asarray(W_proj, dtype=np.float32)
    b_proj = np.asarray(b_proj, dtype=np.float32)

    mask = np.triu(np.ones((128, 128), np.float32)).astype(ml_dtypes.bfloat16)

    in_maps = []
    for c in range(8):
        b = c // 2
        g = c % 2
        cols = slice(g * GC, (g + 1) * GC)
        bq = -2.0 * b_attn[0 * C:1 * C][cols]
        bkv = b_attn[1 * C:2 * C][cols]
        in_maps.append({
            "xT": np.ascontiguousarray(x[b].T),
            "Wq": np.ascontiguousarray(-2.0 * W_attn[:, 0 * C:1 * C][:, cols]),
            "Wk": np.ascontiguousarray(W_attn[:, 1 * C:2 * C][:, cols]),
            "Wv": np.ascontiguousarray(W_attn[:, 2 * C:3 * C][:, cols]),
            "bq_col": np.ascontiguousarray(bq.reshape(4, 128).T),
            "bk_col": np.ascontiguousarray(bkv.reshape(4, 128).T),
            "bk": bkv.reshape(1, GC).copy(),
            "bv": b_attn[2 * C:3 * C][cols].reshape(1, GC).copy(),
            "Wp": np.ascontiguousarray(
                W_proj[g * GC:(g + 1) * GC, :]).astype(ml_dtypes.bfloat16),
            "trimask": mask,
        })

    _last_in_maps = in_maps
    nc = _get_program()
    LAST_RESULTS = run_bass_kernel_spmd(nc, in_maps, core_ids=list(range(8)))

    out = np.empty((B, T, C), np.float32)
    for b in range(B):
        out[b] = (LAST_RESULTS.results[2 * b]["out"]
                  + LAST_RESULTS.results[2 * b + 1]["out"] + b_proj)
    return out



# revision 7
# speedup vs baseline: 1.8826x; 1.8826x over previous
"""Causal Gaussian-kernel self-attention on 8 TRN2 NeuronCores (v2, bf16).

Reference computation (per batch b):
    qkv = x @ W_attn + b_attn ; q,k,v heads of 64 dims
    scores = exp(-(|q|^2 + |k|^2 - 2 q.k) / (2*sqrt(64))), causal-masked, NO softmax
    y = scores @ v ; out = y @ W_proj + b_proj

Sharding: core c -> batch b = c//2, head-group g = c%2 (8 heads each).
Per core the score factors:  exp(q.k/8) * exp(-|q|^2/16) * exp(-|k|^2/16)
  - exp(-|k|^2/16) folded into v (per-key scale)
  - exp(-|q|^2/16) folded into the y^T PSUM->SBUF copy (per-query scale)

v2 changes vs the fp32r baseline (which measured ~500us of PE busy at a
mostly-throttled clock):
  - all matmul operands bf16 (xT, W, q/k packs, v, y, Wp): FWL halves
    LDWEIGHTS, DMA/SBUF halve, and lower PE power should reduce the HAM/SW
    clock throttle that kept the fp32r kernel at ~1.2 GHz.
  - the k-natural GEMM (74 kcyc, used only for |k|^2) is gone: |k|^2 now
    comes from k_pack via a DVE square + block-diagonal-ones matmul
    (8 kcyc), exp'd on ACT in [2,512] chunks into an [8,T] row tile, then
    16 tiny PE transposes -> f_col [128, 16*8] for the v fold.
  - |q|^2 via one block-diag ones matmul per (pair, half) (contraction 128)
    instead of two tile_position'd 64-row matmuls; single [128,1024] exp.
  - attention S tiles are 512-col chunks with both heads of a pair packed
    side by side in one [128, 1024] PSUM tile -> one ACT exp instruction
    covers both heads (160 exp instructions instead of 192+, ACT is the
    second-busiest engine at ~116us of pure element throughput).
  - causal mask multiplies moved to the otherwise-idle GPSIMD engine.
  - warmup matmuls on a zero tile fill the initial input-DMA window so the
    PE enters the kernel already at K=8/8.

Layouts (per core):
  xT      8x(128,2048) x[b]^T bf16, resident
  q_pack  4x(128,2048) head-pair q^T rows, values -2*(x@Wq+bq), bf16
  k_pack  4x(128,2048) head-pair k^T rows, bf16
  v_sb    16x(128,512) v natural * exp(-|k|^2/16), bf16
  s^T     per k-tile 512-chunk (128 k-rows, [hh0 | hh1]) exact-causal, bf16
  y^T     4x(128,2048) head-pair packed, bf16

Host side: the two head-group cores of one batch are summed (the c_proj
row-parallel all-reduce) + b_proj.
"""

import math
import os
from contextlib import ExitStack

import numpy as np
import ml_dtypes

import concourse.bass as bass
import concourse.mybir as mybir
import concourse.tile as tile
from concourse.vector_clock import ScopedClock, VectorClock
from concourse.bass_utils import run_bass_kernel_spmd

F32 = mybir.dt.float32
BF16 = mybir.dt.bfloat16
AF = mybir.ActivationFunctionType
ALU = mybir.AluOpType

B, T, C, H = 4, 2048, 1024, 16
HD = C // H          # 64
HG = H // 2          # 8 heads per core
GC = HG * HD         # 512
NT = T // 128        # 16
NKC = C // 128       # 8
SCALE = -1.0 / (2.0 * math.sqrt(HD))   # -1/16
WU_MM = 28           # warmup matmuls to cover the input-DMA window

LAST_RESULTS = None
_last_in_maps = None


class _TC(tile.TileContext):
    """Tail barrier emitting one NOP per proc tick; this walrus build
    accepts only a single sync wait per instruction."""

    def _drain_and_barrier(self, tick_clock, wait_clock):
        gc = tick_clock.global_clock
        for proc in range(len(gc)):
            if gc[proc] <= 0:
                continue
            vc = VectorClock()
            vc.require_at_least(proc, gc[proc])
            nop_inst = self.nc.sync.nop(nofuse=True)
            wait_clock.add_sem_waits(nop_inst.ins, ScopedClock({None: vc}))
        self.nc.sync.drain()
        self.nc.all_engine_barrier()
        assert self.sems is not None
        popped = self.nc._tile_sem_poison_stack.pop()
        assert popped is self._sem_poison
        self.nc.clear_and_free_semaphores(list(self.sems.allocated().values()))
        self.nc.all_engine_barrier()


def _split_sync_waits(nc, keep=1):
    """Move excess per-instruction sem waits onto NOPs inserted just before,
    same engine stream (walrus here rejects >1 sync wait per instruction)."""
    for f in nc.m.functions:
        for bb in f.blocks:
            out = []
            changed = False
            for inst in bb.instructions:
                si = inst.sync_info
                waits = list(si.on_wait) if (si is not None and si.on_wait) else []
                if len(waits) > keep:
                    changed = True
                    for w in waits[:-keep]:
                        nop = mybir.InstNoOp(
                            name=f"I-wsplit-{nc.next_id()}", ins=[], outs=[]
                        )
                        nop.engine = inst.engine
                        nop.sync_info = mybir.SyncInfo(on_wait=[w], on_update=[])
                        out.append(nop)
                    ups = list(si.on_update) if si.on_update else []
                    inst.sync_info = mybir.SyncInfo(
                        on_wait=waits[-keep:], on_update=ups
                    )
                out.append(inst)
            if changed:
                bb.instructions = out


def _ap3(base, mid_stride, mid_n, inner_n):
    """[128, mid_n, inner_n] view of a 2-D AP with the given middle stride."""
    return bass.AP(
        base.tensor, base.offset,
        [list(base.ap[0]), [mid_stride, mid_n], [1, inner_n]],
    )


def _chunks(q0, q_hi):
    """512-grid-aligned [a0, a1) chunks covering [q0, q_hi)."""
    out = []
    a0 = q0
    while a0 < q_hi:
        a1 = min((a0 // 512 + 1) * 512, q_hi)
        out.append((a0, a1))
        a0 = a1
    return out


def _build_program():
    nc = bass.Bass(target_bir_lowering=False, trn_type="TRN2", debug=False)

    xT_d = nc.dram_tensor("xT", [C, T], BF16, kind="ExternalInput").ap()
    Wq_d = nc.dram_tensor("Wq", [C, GC], BF16, kind="ExternalInput").ap()
    Wk_d = nc.dram_tensor("Wk", [C, GC], BF16, kind="ExternalInput").ap()
    Wv_d = nc.dram_tensor("Wv", [C, GC], BF16, kind="ExternalInput").ap()
    bqc_d = nc.dram_tensor("bq_col", [128, 4], F32, kind="ExternalInput").ap()
    bkc_d = nc.dram_tensor("bk_col", [128, 4], F32, kind="ExternalInput").ap()
    bv_d = nc.dram_tensor("bv", [1, GC], BF16, kind="ExternalInput").ap()
    Wp_d = nc.dram_tensor("Wp", [GC, C], BF16, kind="ExternalInput").ap()
    mask_d = nc.dram_tensor("trimask", [128, 128], BF16, kind="ExternalInput").ap()
    id_d = nc.dram_tensor("ident", [128, 128], BF16, kind="ExternalInput").ap()
    out_d = nc.dram_tensor("out", [T, C], F32, kind="ExternalOutput").ap()

    with _TC(nc) as tc, ExitStack() as ctx:
        res = ctx.enter_context(tc.tile_pool(name="res", bufs=1))
        ps = ctx.enter_context(tc.tile_pool(name="ps", bufs=4, space="PSUM"))

        def big(nm):
            return ps.tile([128, 1024], F32, tag="big", name=nm)

        # ---- resident small tensors ----
        mask = res.tile([128, 128], BF16, tag="mask")
        nc.sync.dma_start(mask[:], mask_d[:])
        ident = res.tile([128, 128], BF16, tag="ident")
        nc.sync.dma_start(ident[:], id_d[:])
        bqc = res.tile([128, 4], F32, tag="bqc")
        nc.sync.dma_start(bqc[:], bqc_d[:])
        bkc = res.tile([128, 4], F32, tag="bkc")
        nc.sync.dma_start(bkc[:], bkc_d[:])
        bv = res.tile([1, GC], BF16, tag="bv")
        nc.sync.dma_start(bv[:], bv_d[:])
        ones_r = res.tile([1, 128], BF16, tag="ones_r")
        nc.vector.memset(ones_r[:], 1.0)
        # block-diag 0.25 for |q~|^2/4 partition-reduce (q~ = -2q)
        ones_q = res.tile([128, 128], BF16, tag="ones_q")
        nc.vector.memset(ones_q[:], 0.0)
        nc.vector.memset(ones_q[0:64, 0:64], 0.25)
        nc.vector.memset(ones_q[64:128, 64:128], 0.25)
        # block-column ones for |k|^2 partition-reduce
        ones_k2 = res.tile([128, 2], BF16, tag="ones_k2")
        nc.vector.memset(ones_k2[:], 0.0)
        nc.vector.memset(ones_k2[0:64, 0:1], 1.0)
        nc.vector.memset(ones_k2[64:128, 1:2], 1.0)
        wu = res.tile([128, 512], BF16, tag="wu")
        nc.vector.memset(wu[:], 0.0)
        # f8[32p+hh, t] = exp(scale*|k|^2) per head row (32-aligned partition
        # bases -- engines cannot address odd partition starts); f_colw holds
        # the transposed [128, tt*128 + 32p+hh] columns.
        f8 = res.tile([128, T], BF16, tag="f8")
        nc.vector.memset(f8[:], 0.0)
        f_colw = res.tile([128, T], BF16, tag="f_colw")

        # ---- resident big tensors ----
        xt_pool = ctx.enter_context(tc.tile_pool(name="xt", bufs=1))
        xT = []
        for kc in range(NKC):
            xt_t = xt_pool.tile([128, T], BF16, tag=f"x{kc}", name=f"xT{kc}")
            nc.sync.dma_start(xt_t[:], xT_d[kc * 128:(kc + 1) * 128, :])
            xT.append(xt_t)
        qk = ctx.enter_context(tc.tile_pool(name="qk", bufs=1))
        q_pack = [qk.tile([128, T], BF16, tag=f"q{p}", name=f"q_pack{p}")
                  for p in range(4)]
        k_pack = [qk.tile([128, T], BF16, tag=f"k{p}", name=f"k_pack{p}")
                  for p in range(4)]
        vt = ctx.enter_context(tc.tile_pool(name="vt", bufs=1))
        v_sb = [vt.tile([128, GC], BF16, tag=f"v{t}", name=f"v_sb{t}")
                for t in range(NT)]
        yp = ctx.enter_context(tc.tile_pool(name="yp", bufs=1))
        y_sb = [yp.tile([128, T], BF16, tag=f"y{p}", name=f"y_sb{p}")
                for p in range(4)]
        q2p = ctx.enter_context(tc.tile_pool(name="q2p", bufs=1))
        q2eh = [q2p.tile([128, T], BF16, tag=f"e{p}", name=f"q2e{p}")
                for p in range(4)]
        wpp = ctx.enter_context(tc.tile_pool(name="wpp", bufs=1))
        wp = [wpp.tile([128, C], BF16, tag=f"wp{p}", name=f"wp{p}")
              for p in range(4)]
        for p in range(4):
            nc.sync.dma_start(wp[p][:], Wp_d[p * 128:(p + 1) * 128, :])

        # ---- PE warmup while input DMAs land ----
        pwu = big("pwu")
        for i in range(WU_MM):
            nc.tensor.matmul(pwu[:, 0:512], wu[:, 0:128], wu[:, 0:512],
                             start=True, stop=True)

        with tc.tile_pool(name="wst", bufs=4) as wst, \
             tc.tile_pool(name="sqk", bufs=2) as sqk, \
             tc.tile_pool(name="wr", bufs=1) as wr:

            # ======== q^T/k^T GEMMs ========
            def emit_qk(p8):
                w_d = Wq_d if p8 < 4 else Wk_d
                bcol = bqc if p8 < 4 else bkc
                m0 = 128 * (p8 % 4)
                dst = q_pack[p8 % 4] if p8 < 4 else k_pack[p8 % 4]
                bigs = [big(f"psqk{p8}_{n2}") for n2 in range(2)]
                for kc in range(NKC):
                    wti = wst.tile([128, 128], BF16, tag="w", name=f"w{p8}_{kc}")
                    nc.sync.dma_start(
                        wti[:], w_d[kc * 128:(kc + 1) * 128, m0:m0 + 128])
                    for n in range(4):
                        nc.tensor.matmul(
                            bigs[n // 2][:, (n % 2) * 512:(n % 2) * 512 + 512],
                            wti[:], xT[kc][:, n * 512:(n + 1) * 512],
                            start=(kc == 0), stop=(kc == NKC - 1))
                for n2 in range(2):
                    nc.vector.tensor_scalar_add(
                        dst[:, n2 * 1024:(n2 + 1) * 1024], bigs[n2][:],
                        bcol[:, (p8 % 4):(p8 % 4) + 1])

            # k pairs first: |k|^2 chain feeds the v fold
            for p in range(4):
                emit_qk(4 + p)
                sq = sqk.tile([128, T], BF16, tag="sqk", name=f"sqk{p}")
                nc.vector.tensor_mul(sq[:], k_pack[p][:], k_pack[p][:])
                pk2 = big(f"pk2_{p}")
                for c in range(4):
                    bp = 32 * (c // 2)
                    co = 512 * (c % 2)
                    nc.tensor.matmul(
                        pk2[bp:bp + 2, co:co + 512], ones_k2[:, 0:2],
                        sq[:, c * 512:(c + 1) * 512],
                        start=True, stop=True, tile_position=(0, bp))
                for c in range(4):
                    bp = 32 * (c // 2)
                    co = 512 * (c % 2)
                    nc.scalar.activation(
                        f8[32 * p:32 * p + 2, c * 512:(c + 1) * 512],
                        pk2[bp:bp + 2, co:co + 512], AF.Exp, scale=SCALE)

            emit_qk(0)
            prio_mark = tc.cur_priority

            # f8 -> f_colw via full 128x128 PE transposes (only columns
            # 32p+hh of each tt block carry data; the rest is never read)
            ptr = big("ptr")
            ptb = ptr[:].bitcast(BF16)
            for tt in range(NT):
                nc.tensor.transpose(
                    ptb[:, tt * 128:(tt + 1) * 128],
                    f8[:, tt * 128:(tt + 1) * 128], ident[:])
            nc.vector.tensor_copy(f_colw[:], ptb[:])

            # ======== v-natural GEMM, fold f_col, cast bf16 ========
            wv_r = [wr.tile([128, GC], BF16, tag=f"wr{kc}", name=f"wvr{kc}")
                    for kc in range(NKC)]
            for kc in range(NKC):
                nc.sync.dma_start(wv_r[kc][:], Wv_d[kc * 128:(kc + 1) * 128, :])
            for tt in range(NT):
                pv = big(f"pv{tt}")
                nc.tensor.matmul(pv[:, 0:GC], ones_r[0:1, 0:128], bv[0:1, :],
                                 start=True, stop=False)
                for kc in range(NKC):
                    nc.tensor.matmul(pv[:, 0:GC],
                                     xT[kc][:, tt * 128:tt * 128 + 128],
                                     wv_r[kc][:], start=False,
                                     stop=(kc == NKC - 1))
                fb = f_colw[:, tt * 128:(tt + 1) * 128]
                fb_b = bass.AP(fb.tensor, fb.offset,
                               [list(fb.ap[0]), [32, 4], [1, 2], [0, HD]])
                nc.vector.tensor_tensor(
                    v_sb[tt].rearrange("p (a b d) -> p a b d", a=4, b=2),
                    pv[:, 0:GC].rearrange("p (a b d) -> p a b d", a=4, b=2),
                    fb_b, op=ALU.mult)

            for p8 in (1, 2, 3):
                emit_qk(p8)

        # ================= attention (elevated priority) =================
        with tc.tile_pool(name="sqq", bufs=2) as sqq, \
             tc.tile_pool(name="ssb", bufs=3) as ssb, \
             tc.tile_pool(name="osb", bufs=2) as osb:

            with tc.high_priority(offset=max(0, tc.cur_priority - prio_mark)):
                for half in range(2):
                    q_lo, q_hi = 1024 * half, 1024 * (half + 1)
                    for p in range(4):
                        # q2e for this (pair, half)
                        sq_q = sqq.tile([128, 1024], BF16, tag="sqq",
                                        name=f"sqq{p}_{half}")
                        nc.vector.tensor_mul(sq_q[:], q_pack[p][:, q_lo:q_hi],
                                             q_pack[p][:, q_lo:q_hi])
                        pq2 = big(f"pq2_{p}{half}")
                        for j in range(2):
                            nc.tensor.matmul(
                                pq2[:, j * 512:(j + 1) * 512], ones_q[:],
                                sq_q[:, j * 512:(j + 1) * 512],
                                start=True, stop=True)
                        nc.scalar.activation(q2eh[p][:, q_lo:q_hi], pq2[:],
                                             AF.Exp, scale=SCALE)

                        y_ps = big(f"yps{p}_{half}")
                        kt_last = 8 * half + 7
                        for kt in range(kt_last + 1):
                            q0 = max(128 * kt, q_lo)
                            for (a0, a1) in _chunks(q0, q_hi):
                                w = a1 - a0
                                s_ps = big(f"sps{p}_{half}_{kt}_{a0}")
                                for hh in range(2):
                                    nc.tensor.matmul(
                                        s_ps[:, hh * 512:hh * 512 + w],
                                        k_pack[p][hh * 64:hh * 64 + 64,
                                                  kt * 128:kt * 128 + 128],
                                        q_pack[p][hh * 64:hh * 64 + 64,
                                                  a0:a1],
                                        start=True, stop=True,
                                        tile_position=(hh * 64, 0))
                                s_sb = ssb.tile([128, 1024], BF16, tag="s",
                                                name=f"ssb{p}_{half}_{kt}_{a0}")
                                nc.scalar.activation(
                                    _ap3(s_sb[:], 512, 2, w),
                                    _ap3(s_ps[:], 512, 2, w),
                                    AF.Exp, scale=SCALE)
                                if a0 == 128 * kt and 128 * kt >= q_lo:
                                    mb = mask[:]
                                    nc.gpsimd.tensor_tensor(
                                        _ap3(s_sb[:], 512, 2, 128),
                                        _ap3(s_sb[:], 512, 2, 128),
                                        bass.AP(mb.tensor, mb.offset,
                                                [list(mb.ap[0]), [0, 2],
                                                 [1, 128]]),
                                        op=ALU.mult)
                                for hh in range(2):
                                    h = 2 * p + hh
                                    nc.tensor.matmul(
                                        y_ps[hh * 64:hh * 64 + 64,
                                             a0 - q_lo:a1 - q_lo],
                                        v_sb[kt][:, h * HD:h * HD + HD],
                                        s_sb[:, hh * 512:hh * 512 + w],
                                        start=(kt == 0), stop=(kt == kt_last),
                                        tile_position=(0, hh * 64))
                        nc.vector.tensor_tensor(
                            y_sb[p][:, q_lo:q_hi], y_ps[:],
                            q2eh[p][:, q_lo:q_hi], op=ALU.mult)

                    # ---- c_proj for this T-half ----
                    for tt in range(8 * half, 8 * half + 8):
                        po = big(f"po{tt}")
                        for p4 in range(4):
                            for n2 in range(2):
                                nc.tensor.matmul(
                                    po[:, n2 * 512:(n2 + 1) * 512],
                                    y_sb[p4][:, tt * 128:(tt + 1) * 128],
                                    wp[p4][:, n2 * 512:(n2 + 1) * 512],
                                    start=(p4 == 0), stop=(p4 == 3))
                        o_sb = osb.tile([128, C], F32, tag="o", name=f"osb{tt}")
                        nc.vector.tensor_copy(o_sb[:], po[:])
                        nc.sync.dma_start(out_d[tt * 128:(tt + 1) * 128, :],
                                          o_sb[:])

    _split_sync_waits(nc)
    return nc


_NC_CACHE = None


def _get_program():
    global _NC_CACHE
    if _NC_CACHE is None:
        _NC_CACHE = _build_program()
    return _NC_CACHE


def kernel(x, W_attn, b_attn, W_proj, b_proj, n_head):
    global LAST_RESULTS, _last_in_maps
    assert int(n_head) == H
    x = np.asarray(x, dtype=np.float32)
    W_attn = np.asarray(W_attn, dtype=np.float32)
    b_attn = np.asarray(b_attn, dtype=np.float32)
    W_proj = np.asarray(W_proj, dtype=np.float32)
    b_proj = np.asarray(b_proj, dtype=np.float32)

    bf = ml_dtypes.bfloat16
    mask = np.triu(np.ones((128, 128), np.float32)).astype(bf)
    ident = np.eye(128, dtype=np.float32).astype(bf)

    in_maps = []
    for c in range(8):
        b = c // 2
        g = c % 2
        cols = slice(g * GC, (g + 1) * GC)
        bq = -2.0 * b_attn[0 * C:1 * C][cols]
        bk = b_attn[1 * C:2 * C][cols]
        in_maps.append({
            "xT": np.ascontiguousarray(x[b].T).astype(bf),
            "Wq": np.ascontiguousarray(
                -2.0 * W_attn[:, 0 * C:1 * C][:, cols]).astype(bf),
            "Wk": np.ascontiguousarray(
                W_attn[:, 1 * C:2 * C][:, cols]).astype(bf),
            "Wv": np.ascontiguousarray(
                W_attn[:, 2 * C:3 * C][:, cols]).astype(bf),
            "bq_col": np.ascontiguousarray(bq.reshape(4, 128).T),
            "bk_col": np.ascontiguousarray(bk.reshape(4, 128).T),
            "bv": b_attn[2 * C:3 * C][cols].reshape(1, GC).astype(bf),
            "Wp": np.ascontiguousarray(
                W_proj[g * GC:(g + 1) * GC, :]).astype(bf),
            "trimask": mask,
            "ident": ident,
        })

    _last_in_maps = in_maps
    nc = _get_program()
    LAST_RESULTS = run_bass_kernel_spmd(nc, in_maps, core_ids=list(range(8)))

    out = np.empty((B, T, C), np.float32)
    for b in range(B):
        out[b] = (LAST_RESULTS.results[2 * b]["out"]
                  + LAST_RESULTS.results[2 * b + 1]["out"] + b_proj)
    return out
